# revision 1
# baseline (speedup 1.0000x reference)
import numpy as np
import jax
import jax.numpy as jnp
from functools import partial

# Problem constants (nn_BallCritic_88673894793691)
BS, N, HID, EMB, K, NUM_CLASSES = 1024, 64, 128, 64, 8, 3
NCORES = 8

_COMPILED = {}


def _fold_branch(p):
    """Host-side parameter preprocessing (numpy, tiny).

    h = [tanh(se) | tanh(ce)], msg1 = [h_i, h_j - h_i] @ m_w1 + m_b1
       = h_i @ (Wa - Wb) + h_j @ Wb + m_b1   with Wa = m_w1[:192], Wb = m_w1[192:]
    tanh(ce) is one of 3 per-class vectors -> fold through Wa/Wb into 3-row tables.
    """
    f32 = lambda a: np.asarray(a, np.float32)
    sp_w1, sp_b1 = f32(p['sp_w1']), f32(p['sp_b1'])
    sp_w2, sp_b2 = f32(p['sp_w2']), f32(p['sp_b2'])
    emb, em_w, em_b = f32(p['emb']), f32(p['em_w']), f32(p['em_b'])
    m_w1, m_b1 = f32(p['m_w1']), f32(p['m_b1'])
    m_w2, m_b2 = f32(p['m_w2']), f32(p['m_b2'])
    t_w1, t_b1 = f32(p['t_w1']), f32(p['t_b1'])
    t_w2, t_b2 = f32(p['t_w2']), f32(p['t_b2'])

    ctab = np.tanh(np.tanh(emb) @ em_w + em_b)          # [3, EMB] = tanh(ce) per class
    Wa, Wb = m_w1[:HID + EMB], m_w1[HID + EMB:]         # [192,128] each
    Wa_se, Wb_se = Wa[:HID], Wb[:HID]                   # tanh(se) paths
    Wa_ce, Wb_ce = Wa[HID:], Wb[HID:]                   # tanh(ce) paths
    return dict(
        sp_w1=sp_w1, sp_b1=sp_b1, sp_w2=sp_w2, sp_b2=sp_b2,
        WA_se=(Wa_se - Wb_se), WB_se=Wb_se,
        ccA=(ctab @ (Wa_ce - Wb_ce)) + m_b1,            # [3,128] class term of A (+bias)
        ccB=ctab @ Wb_ce,                               # [3,128] class term of B
        m_w2=m_w2, m_b2=m_b2,
        t_w1=t_w1, t_b1=t_b1, t_w2=t_w2, t_b2=t_b2,
    )


def _branch_local(pos, oh3, spatial, nbr_oh, W):
    """One critic branch for a local shard. pos:[b,N,2] oh3:[b,N,3]
    spatial:[b,N,6] nbr_oh: list of K one-hot [b,N,N]. Returns [b,N]."""
    b = spatial.shape[0]
    x = spatial.reshape(b * N, 6)
    ts = jnp.tanh(jnp.tanh(x @ W['sp_w1'] + W['sp_b1']) @ W['sp_w2'] + W['sp_b2'])
    oh = oh3.reshape(b * N, 3)
    A = (ts @ W['WA_se'] + oh @ W['ccA']).reshape(b, N, HID)
    B = (ts @ W['WB_se'] + oh @ W['ccB']).reshape(b, N, HID)
    M = None
    for k in range(K):
        Bk = jnp.matmul(nbr_oh[k], B)                   # [b,N,HID] gather via one-hot
        mk = jnp.tanh(A + Bk) @ W['m_w2']               # [b,N,HID]
        M = mk if M is None else jnp.maximum(M, mk)
    xh = jnp.tanh(M + W['m_b2']).reshape(b * N, HID)
    q = jnp.tanh(xh @ W['t_w1'] + W['t_b1']) @ W['t_w2'] + W['t_b2']
    return q.reshape(b, N)


def _forward_local(state, action, tar_scores, W1, W2):
    b = state.shape[0]
    st = state.reshape(b, N, 3)
    pos = st[..., :2]
    cats = st[..., 2]
    oh3 = (cats[..., None] == jnp.arange(3, dtype=jnp.float32)).astype(jnp.float32)
    spatial = jnp.concatenate(
        [pos, action.reshape(b, N, 2), jnp.tanh(tar_scores.reshape(b, N, 2))], axis=-1)

    # kNN: exact top-K smallest d2 (no self), jax.top_k tie-break (lowest index first)
    d2 = jnp.sum((pos[:, :, None, :] - pos[:, None, :, :]) ** 2, axis=-1)
    d2 = d2 + jnp.eye(N, dtype=d2.dtype) * 1e10
    nbr_oh = []
    for _ in range(K):
        m = jnp.min(d2, axis=-1, keepdims=True)
        eq = (d2 == m).astype(jnp.float32)
        first = (jnp.cumsum(eq, axis=-1) <= 1.0) * eq   # lowest index among ties
        nbr_oh.append(first)
        d2 = d2 + first * 1e10
    q1 = _branch_local(pos, oh3, spatial, nbr_oh, W1)
    q2 = _branch_local(pos, oh3, spatial, nbr_oh, W2)
    return jnp.stack([q1, q2])                          # [2,b,N]


def _get_compiled():
    key = 'fn'
    if key in _COMPILED:
        return _COMPILED[key]
    from jax.sharding import Mesh, PartitionSpec as P
    from jax.experimental.shard_map import shard_map

    devs = jax.devices()[:NCORES]
    mesh = Mesh(np.array(devs), ('x',))
    wspec = dict(
        sp_w1=P(), sp_b1=P(), sp_w2=P(), sp_b2=P(), WA_se=P(), WB_se=P(),
        ccA=P(), ccB=P(), m_w2=P(), m_b2=P(), t_w1=P(), t_b1=P(), t_w2=P(), t_b2=P())
    fn = jax.jit(shard_map(
        _forward_local, mesh=mesh,
        in_specs=(P('x'), P('x'), P('x'), wspec, wspec),
        out_specs=P(None, 'x'), check_rep=False))
    _COMPILED[key] = fn
    return fn


def kernel(state, action, tar_scores, params):
    state = np.asarray(state, np.float32)
    action = np.asarray(action, np.float32)
    tar = np.asarray(tar_scores, np.float32).reshape(BS, N * 2)
    W1 = _fold_branch(params['q1'])
    W2 = _fold_branch(params['q2'])
    try:
        fn = _get_compiled()
        out = fn(state, action, tar, W1, W2)
        return np.asarray(out, np.float32)
    except Exception:
        # Fallback: single-device jit (still on trn2, core 0)
        key = 'fn1'
        if key not in _COMPILED:
            _COMPILED[key] = jax.jit(_forward_local)
        out = _COMPILED[key](state, action, tar, W1, W2)
        return np.asarray(out, np.float32)
